# revision 29
# baseline (speedup 1.0000x reference)
"""DCGRU classifier kernel for Trainium2 (8 NeuronCores, batch-data-parallel).

v4 layout strategy (per core, B_loc=4 batch items):
  - All matmul operands bf16 (4x PE throughput vs fp32); PSUM accumulates fp32.
  - Activations FEATURE-major: tiles are (features, batch*node).
  - gconv reordered as  z@(W0-W2) + S@(z@W1) + (2S^2)@(z@W2).
  - Joint state tile zT1 = [h1(0:U); h2(U:2U)] (ping-pong pair):
      * L0's update writes h1 directly at partition 0 (no fanout copies),
      * L1's gate reads the whole tile contiguously,
      * L0's gate reads h1 via a K=64 partial matmul + a K=16 x-part matmul
        at base partition 64 (tile_position), accumulated in PSUM.
  - L1's gate emits val1=[u; r] (column-flipped weights) so all of L1's
    elementwise work sits at partitions 64:128 and needs no alignment copies;
    L1's candidate uses split-K matmuls (h1-part at base 0, rh2-part at
    base 64) so no zc tile for L1 exists.
  - Software pipeline: L1 lags L0 by one step; L1's gate is split across the
    block boundary (projections at the end of block t, diffusion+activation
    at the start of block t+1) so the PE always has ready work.
  - h' = u*h + (1-u)*c computed as  e=(u*h) [gpsimd, off-spine],
    m=(1-u)*c, h'=e+m;  1-u comes from a second sigmoid with scale=-1.
  - h2_t is DMA'd to DRAM every step; host picks t = seq_len-1 per item.
  - t_steps = max(seq_lengths); final relu->fc->maxpool tail on host.
"""

import sys

import numpy as np
import ml_dtypes

sys.path.insert(0, "/opt/trn_rl_repo")

import concourse.bass as bass
import concourse.bacc as bacc
import concourse.mybir as mybir
from concourse.bass_utils import run_bass_kernel_spmd
from concourse.tile import TileContext

B, T, N, DIN, U, C = 32, 256, 128, 16, 64, 4
NCORES = 8
BL = B // NCORES  # 4 batch items per core
BN = BL * N  # 512
F32 = mybir.dt.float32
BF16 = mybir.dt.bfloat16
BF16_NP = ml_dtypes.bfloat16


# packed bf16 constant blob: (row_count, col_offset, col_count)
def _blob_layout():
    lay = {}
    col = 0

    def seg(key, rows, cols):
        nonlocal col
        lay[key] = (rows, col, cols)
        col += cols

    seg("S_T", N, N)
    seg("S2_T", N, N)
    for l, D in ((0, DIN + U), (1, 2 * U)):
        seg((l, "g12"), D, 4 * U)
        seg((l, "g0"), D, 2 * U)
        seg((l, "c12"), D, 2 * U)
        seg((l, "c0"), D, U)
    # base-partition-0 duplicates of row-slices used as split-K operands
    seg((0, "g12x"), DIN, 4 * U)
    seg((0, "g0x"), DIN, 2 * U)
    seg((1, "c12r"), U, 2 * U)
    seg((1, "c0r"), U, U)
    return lay, col


_BLOB_LAYOUT, BLOB_COLS = _blob_layout()

_NC_CACHE = {}


def _build_nc(t_steps: int):
    nc = bacc.Bacc("TRN2")

    xT_e = nc.declare_dram_parameter("xT", [t_steps, DIN, BN], BF16, isOutput=False)
    blob_e = nc.declare_dram_parameter("blob", [N, BLOB_COLS], BF16, isOutput=False)
    bias_e = nc.declare_dram_parameter("bias", [N, 6], F32, isOutput=False)
    h2seq_e = nc.declare_dram_parameter("h2seq", [t_steps, U, BN], BF16, isOutput=True)

    with TileContext(nc) as tc:
        with (
            tc.tile_pool(name="singles", bufs=1) as singles,
            tc.tile_pool(name="sq", bufs=2) as sq_pool,
            tc.tile_pool(name="sval", bufs=2) as sval_pool,
            tc.tile_pool(name="pq", bufs=2, space="PSUM") as pq_pool,
            tc.tile_pool(name="pqc", bufs=2, space="PSUM") as pqc_pool,
            tc.tile_pool(name="pval", bufs=2, space="PSUM") as pval_pool,
            tc.tile_pool(name="pc", bufs=2, space="PSUM") as pc_pool,
        ):
            # ---- persistent tiles ----
            blob = singles.tile([N, BLOB_COLS], BF16)
            nc.sync.dma_start(out=blob, in_=blob_e[:, :])
            bias = singles.tile([N, 6], F32)
            nc.sync.dma_start(out=bias, in_=bias_e[:, :])

            def wv(key, r0=None, r1=None):
                rows, c0, cols = _BLOB_LAYOUT[key]
                if r0 is None:
                    r0, r1 = 0, rows
                return blob[r0:r1, c0 : c0 + cols]

            st = wv("S_T")
            s2t = wv("S2_T")

            # joint state [h1(0:U); h2(U:2U)], ping-pong; zcT0 = [rh1; x]
            zS_0 = singles.tile([2 * U, BN], BF16)
            zS_1 = singles.tile([2 * U, BN], BF16)
            zS = (zS_0, zS_1)
            zcT0 = singles.tile([DIN + U, BN], BF16)
            x0 = singles.tile([DIN, BN], BF16)  # x at partitions 0:DIN
            rh2t = singles.tile([2 * U, BN], BF16)  # rh2 lives at [U:2U]
            rh2p0 = singles.tile([U, BN], BF16)  # partition-0 copy for lhsT
            nc.vector.memset(zS_0[:, :], 0.0)
            nc.vector.memset(zS_1[:, :], 0.0)

            def gate_project0(cur):
                """L0 gate projections: h1-part (K=64) + x-part (K=16@p64)."""
                h1 = zS[cur]
                pqA = pq_pool.tile([N, 2, 4 * U], F32, tag="pq", name="pqA")
                pqB = pq_pool.tile([N, 2, 4 * U], F32, tag="pq", name="pqB")
                for b in range(BL):
                    dst = pqA[:, b, :] if b < 2 else pqB[:, b - 2, :]
                    nc.tensor.matmul(
                        dst, lhsT=x0[:, b * N : (b + 1) * N],
                        rhs=wv((0, "g12x")),
                        start=True, stop=False,
                    )
                    nc.tensor.matmul(
                        dst, lhsT=h1[0:U, b * N : (b + 1) * N],
                        rhs=wv((0, "g12"), 0, U),
                        start=False, stop=True, skip_group_check=True,
                    )
                pval = pval_pool.tile([2 * U, BN], F32, tag="pval", name="pval")
                nc.tensor.matmul(
                    pval, lhsT=wv((0, "g0x")), rhs=x0,
                    start=True, stop=False,
                )
                nc.tensor.matmul(
                    pval, lhsT=wv((0, "g0"), 0, U), rhs=h1[0:U, :],
                    start=False, stop=False, skip_group_check=True,
                )
                return pqA, pqB, pval

            def gate_project1(cur):
                """L1 gate projections on the joint state (K=128, no split)."""
                z1 = zS[cur]
                pqA = pq_pool.tile([N, 2, 4 * U], F32, tag="pq", name="pqA")
                pqB = pq_pool.tile([N, 2, 4 * U], F32, tag="pq", name="pqB")
                for b in range(BL):
                    dst = pqA[:, b, :] if b < 2 else pqB[:, b - 2, :]
                    nc.tensor.matmul(
                        dst, lhsT=z1[:, b * N : (b + 1) * N], rhs=wv((1, "g12")),
                        start=True, stop=True,
                    )
                pval = pval_pool.tile([2 * U, BN], F32, tag="pval", name="pval")
                nc.tensor.matmul(pval, lhsT=wv((1, "g0")), rhs=z1, start=True, stop=False)
                return pqA, pqB, pval

            def gate_evac(pqA, pqB):
                q12 = sq_pool.tile([N, BL, 4 * U], BF16, tag="q12", name="q12")
                nc.vector.tensor_copy(q12[:, 0:2, :], pqA)
                nc.scalar.copy(q12[:, 2:4, :], pqB)
                return q12

            def gate_diffuse(q12, pval):
                for b in range(BL):
                    blk = pval[:, b * N : (b + 1) * N]
                    nc.tensor.matmul(
                        blk, lhsT=q12[:, b, 0 : 2 * U], rhs=st,
                        start=False, stop=False, skip_group_check=True,
                    )
                    nc.tensor.matmul(
                        blk, lhsT=q12[:, b, 2 * U : 4 * U], rhs=s2t,
                        start=False, stop=(b == BL - 1), skip_group_check=True,
                    )

            def gate_act(l, pval, po):
                """Sigmoids.  po=0: val=[r;u], aux tiles at partitions 0:U.
                po=U: val=[u;r], aux tiles at partitions U:2U (L1)."""
                val = sval_pool.tile([2 * U, BN], BF16, tag="val%d" % l, name="val")
                nc.scalar.activation(
                    val, pval, mybir.ActivationFunctionType.Sigmoid,
                    bias=bias[0 : 2 * U, 3 * l : 3 * l + 1],
                )
                # um = 1-u from the u-half of psum (sigmoid(-x-b))
                upsum = pval[U : 2 * U, :] if po == 0 else pval[0:U, :]
                um = sval_pool.tile([2 * U, BN], BF16, tag="um%d" % l, name="um")
                nc.scalar.activation(
                    um[po : po + U, :], upsum, mybir.ActivationFunctionType.Sigmoid,
                    bias=bias[0:U, 3 * l + 2 : 3 * l + 3], scale=-1.0,
                )
                # aligned copy of u (off the critical path, DMA)
                uval = val[U : 2 * U, :] if po == 0 else val[0:U, :]
                u0 = sval_pool.tile([2 * U, BN], BF16, tag="u0%d" % l, name="u0")
                nc.sync.dma_start(out=u0[po : po + U, :], in_=uval)
                return val, u0, um

            def cand_finish(l, qc_mms, pc_mms, po):
                """qc_mms/pc_mms emit the projection matmuls given psum tiles."""
                pqc = pqc_pool.tile([N, BL, 2 * U], F32, tag="pqc", name="pqc")
                qc_mms(pqc)
                qc = sq_pool.tile([N, BL, 2 * U], BF16, tag="qc", name="qc")
                nc.vector.tensor_copy(qc, pqc)
                pc = pc_pool.tile([U, BN], F32, tag="pc", name="pc")
                pc_mms(pc)
                for b in range(BL):
                    blk = pc[:, b * N : (b + 1) * N]
                    nc.tensor.matmul(
                        blk, lhsT=qc[:, b, 0:U], rhs=st,
                        start=False, stop=False, skip_group_check=True,
                    )
                    nc.tensor.matmul(
                        blk, lhsT=qc[:, b, U : 2 * U], rhs=s2t,
                        start=False, stop=(b == BL - 1), skip_group_check=True,
                    )
                c = sval_pool.tile([2 * U, BN], BF16, tag="c", name="c")
                nc.scalar.activation(
                    c[po : po + U, :], pc, mybir.ActivationFunctionType.Tanh,
                    bias=bias[0:U, 3 * l + 1 : 3 * l + 2],
                )
                return c

            def update_pre(l, u0, h_prev, po):
                """e = u*h_prev; off the spine, emitted early."""
                e = sval_pool.tile([2 * U, BN], BF16, tag="e%d" % l, name="e")
                eng = nc.gpsimd if po == 0 else nc.vector
                eng.tensor_mul(e[po : po + U, :], u0[po : po + U, :], h_prev)
                return e

            def update_post(um, c, e, h_out, po):
                """h_out = e + (1-u)*c  (the on-spine half of the update)."""
                m = sval_pool.tile([2 * U, BN], BF16, tag="m", name="m")
                nc.vector.tensor_mul(
                    m[po : po + U, :], um[po : po + U, :], c[po : po + U, :]
                )
                nc.vector.tensor_add(h_out, e[po : po + U, :], m[po : po + U, :])

            # --- software-pipelined time loop ---
            pend = None  # (t-1, pval1, q12_1) awaiting diffusion + activation
            for t in range(t_steps + 1):
                cur, prv = t % 2, (t + 1) % 2
                if t < t_steps:
                    nc.sync.dma_start(out=zcT0[U : U + DIN, :], in_=xT_e[t])
                    nc.sync.dma_start(out=x0, in_=xT_e[t])

                    # --- B: L0 gate projections + evac (step t) ---
                    pqA0, pqB0, pval0 = gate_project0(prv)
                    q12_0 = gate_evac(pqA0, pqB0)

                # --- Dtail: L1 gate diffusion + activation (step t-1) ---
                if pend is not None:
                    tp, pval1, q12_1 = pend
                    gate_diffuse(q12_1, pval1)
                    val1, u0_1, um1 = gate_act(1, pval1, U)

                    # --- A: L1 candidate for step t-1 ---
                    h2_prev = zS[tp % 2][U : 2 * U, :]
                    e1 = update_pre(1, u0_1, h2_prev, U)
                    nc.vector.tensor_mul(rh2t[U : 2 * U, :], val1[U : 2 * U, :], h2_prev)
                    nc.vector.tensor_copy(rh2p0, rh2t[U : 2 * U, :])

                if t < t_steps:
                    gate_diffuse(q12_0, pval0)

                if pend is not None:
                    def qc1_mms(pqc):
                        for b in range(BL):
                            nc.tensor.matmul(
                                pqc[:, b, :],
                                lhsT=zS[tp % 2][0:U, b * N : (b + 1) * N],
                                rhs=wv((1, "c12"), 0, U),
                                start=True, stop=False,
                            )
                            nc.tensor.matmul(
                                pqc[:, b, :],
                                lhsT=rh2p0[:, b * N : (b + 1) * N],
                                rhs=wv((1, "c12r")),
                                start=False, stop=True, skip_group_check=True,
                            )

                    def pc1_mms(pc):
                        nc.tensor.matmul(
                            pc, lhsT=wv((1, "c0"), 0, U), rhs=zS[tp % 2][0:U, :],
                            start=True, stop=False,
                        )
                        nc.tensor.matmul(
                            pc, lhsT=wv((1, "c0r")), rhs=rh2p0,
                            start=False, stop=False, skip_group_check=True,
                        )

                if t < t_steps:
                    val0, u0_0, um0 = gate_act(0, pval0, 0)

                if pend is not None:
                    c1 = cand_finish(1, qc1_mms, pc1_mms, U)
                    update_post(um1, c1, e1, zS[(tp + 1) % 2][U : 2 * U, :], U)
                    nc.sync.dma_start(
                        out=h2seq_e[tp], in_=zS[(tp + 1) % 2][U : 2 * U, :]
                    )

                if t < t_steps:
                    # --- C: L0 candidate + update (step t) ---
                    h1_prev = zS[prv][0:U, :]
                    e0 = update_pre(0, u0_0, h1_prev, 0)
                    nc.vector.tensor_mul(zcT0[0:U, :], val0[0:U, :], h1_prev)

                    def qc0_mms(pqc):
                        for b in range(BL):
                            nc.tensor.matmul(
                                pqc[:, b, :],
                                lhsT=zcT0[:, b * N : (b + 1) * N],
                                rhs=wv((0, "c12")),
                                start=True, stop=True,
                            )

                    def pc0_mms(pc):
                        nc.tensor.matmul(
                            pc, lhsT=wv((0, "c0")), rhs=zcT0, start=True, stop=False
                        )

                    c0 = cand_finish(0, qc0_mms, pc0_mms, 0)
                    update_post(um0, c0, e0, zS[cur][0:U, :], 0)

                    # --- Dproj: L1 gate projections + evac (step t) ---
                    pqA1, pqB1, pval1n = gate_project1(cur)
                    q12_1n = gate_evac(pqA1, pqB1)
                    pend = (t, pval1n, q12_1n)
                else:
                    pend = None

    nc.compile()
    return nc


def _prep_shared(support, W0_gate, W0_cand, W1_gate, W1_cand,
                 b0_gate, b0_cand, b1_gate, b1_cand):
    f = np.float32
    S = np.asarray(support, f)
    seg = {
        "S_T": np.ascontiguousarray(S.T),
        "S2_T": np.ascontiguousarray((2.0 * (S @ S)).T),
    }
    bias = np.zeros((N, 6), f)
    for l, (Wg, Wc, bg, bc) in enumerate(
        ((W0_gate, W0_cand, b0_gate, b0_cand), (W1_gate, W1_cand, b1_gate, b1_cand))
    ):
        Wg = np.asarray(Wg, f)
        Wc = np.asarray(Wc, f)
        g = [Wg[m::3] for m in range(3)]
        c = [Wc[m::3] for m in range(3)]
        bg = np.asarray(bg, f).reshape(-1)
        if l == 0:
            # device z-layout for layer0 is [h(64); x(16)]
            perm = np.concatenate([np.arange(DIN, DIN + U), np.arange(DIN)])
            g = [gm[perm] for gm in g]
            c = [cm[perm] for cm in c]
        else:
            # layer1: z = [h1; h2] / zc = [h1; rh2] match the reference row
            # order, but the gate OUTPUT is column-flipped to [u; r] so L1's
            # elementwise work can live at partitions U:2U.
            cflip = np.concatenate([np.arange(U, 2 * U), np.arange(U)])
            g = [gm[:, cflip] for gm in g]
            bg = bg[cflip]
        seg[(l, "g12")] = np.concatenate([g[1], g[2]], axis=1)
        seg[(l, "g0")] = g[0] - g[2]
        seg[(l, "c12")] = np.concatenate([c[1], c[2]], axis=1)
        seg[(l, "c0")] = c[0] - c[2]
        if l == 0:
            seg[(0, "g12x")] = seg[(0, "g12")][U : U + DIN]
            seg[(0, "g0x")] = seg[(0, "g0")][U : U + DIN]
        else:
            seg[(1, "c12r")] = seg[(1, "c12")][U : 2 * U]
            seg[(1, "c0r")] = seg[(1, "c0")][U : 2 * U]
        bias[0 : 2 * U, 3 * l] = bg
        bias[0:U, 3 * l + 1] = np.asarray(bc, f).reshape(-1)
        # -u_bias for the (1-u) sigmoid; the u-half sits at rows U:2U for
        # layer0 ([r;u]) and rows 0:U for layer1 ([u;r]).
        bias[0:U, 3 * l + 2] = -(bg[U:] if l == 0 else bg[:U])
    blob = np.zeros((N, BLOB_COLS), BF16_NP)
    for key, (rows, c0, cols) in _BLOB_LAYOUT.items():
        a = seg[key]
        assert a.shape == (rows, cols), (key, a.shape, rows, cols)
        blob[:rows, c0 : c0 + cols] = a.astype(BF16_NP)
    return {"blob": blob, "bias": bias}


def run_cores(inputs, t_steps=T, trace=False):
    """Build in_maps, run the SPMD kernel, return per-core h2 sequences."""
    input_seq = np.asarray(inputs["input_seq"], np.float32)
    shared = _prep_shared(
        inputs["support"], inputs["W0_gate"], inputs["W0_cand"],
        inputs["W1_gate"], inputs["W1_cand"],
        inputs["b0_gate"], inputs["b0_cand"], inputs["b1_gate"], inputs["b1_cand"],
    )
    in_maps = []
    for k in range(NCORES):
        xs = input_seq[k * BL : (k + 1) * BL, :t_steps]  # (BL, t, N, DIN)
        xT = np.ascontiguousarray(
            np.transpose(xs, (1, 3, 0, 2)).reshape(t_steps, DIN, BN)
        ).astype(BF16_NP)
        in_maps.append(dict(shared, xT=xT))
    if t_steps not in _NC_CACHE:
        _NC_CACHE[t_steps] = _build_nc(t_steps)
    nc = _NC_CACHE[t_steps]
    res = run_bass_kernel_spmd(nc, in_maps, list(range(NCORES)), trace=trace)
    return res


def finish_host(results, inputs):
    """Host tail: pick h2 at t=seq_len-1, then relu -> fc -> node max-pool."""
    W_fc = np.asarray(inputs["W_fc"], np.float32)
    b_fc = np.asarray(inputs["b_fc"], np.float32)
    seq = np.asarray(inputs["seq_lengths"]).astype(np.int64)
    out = np.empty((B, C), np.float32)
    for k in range(NCORES):
        h2seq = results[k]["h2seq"]  # (t_steps, U, BN) bf16
        t_steps = h2seq.shape[0]
        for b in range(BL):
            tb = int(min(seq[k * BL + b] - 1, t_steps - 1))
            blk = np.asarray(h2seq[tb, :, b * N : (b + 1) * N], np.float32).T  # (N, U)
            logits = np.maximum(blk, 0.0) @ W_fc + b_fc  # (N, C)
            out[k * BL + b] = logits.max(axis=0)
    return out


def kernel(**inputs):
    seq = np.asarray(inputs["seq_lengths"]).astype(np.int64)
    t_steps = int(min(T, max(1, int(seq.max()))))
    res = run_cores(inputs, t_steps=t_steps)
    return finish_host(res.results, inputs)


# revision 30
# speedup vs baseline: 1.2279x; 1.2279x over previous
"""DCGRU classifier kernel for Trainium2 (8 NeuronCores, batch-data-parallel).

v2 layout strategy (per core, B_loc=4 batch items):
  - All matmul operands bf16 (4x PE throughput vs fp32); PSUM accumulates fp32.
  - Activations FEATURE-major: tiles are (features, batch*node) so the
    recurrent state, gates and candidate need no transposes.
  - gconv reordered as  z@(W0-W2) + S@(z@W1) + (2S^2)@(z@W2); S^T and (2S^2)^T
    are host-precomputed so the two diffusion terms are independent.
  - Gate projections combined: z@[W1|W2] -> (N, 256) per batch, one matmul.
  - One sigmoid for [r|u] (bias per-partition, stacked), no 1-u activation:
    state update is h' = c + u*(h-c)  (3 DVE ops).
  - h2 state ping-pongs between two tiles; every step h2_t is DMA'd to DRAM,
    host picks t = seq_len-1 per item (replaces in-loop predicated select).
  - t_steps = max(seq_lengths) (host-side, steps beyond it are never read).
  - Final relu->fc->maxpool tail done on host (tiny).
"""

import sys

import numpy as np
import ml_dtypes

sys.path.insert(0, "/opt/trn_rl_repo")

import concourse.bass as bass
import concourse.bacc as bacc
import concourse.mybir as mybir
from concourse.bass_utils import run_bass_kernel_spmd
from concourse.tile import TileContext

B, T, N, DIN, U, C = 32, 256, 128, 16, 64, 4
NCORES = 8
BL = B // NCORES  # 4 batch items per core
BN = BL * N  # 512
F32 = mybir.dt.float32
BF16 = mybir.dt.bfloat16
BF16_NP = ml_dtypes.bfloat16


# packed bf16 constant blob: (row_count, col_offset, col_count)
def _blob_layout():
    lay = {}
    col = 0

    def seg(key, rows, cols):
        nonlocal col
        lay[key] = (rows, col, cols)
        col += cols

    seg("S_T", N, N)
    seg("S2_T", N, N)
    for l, D in ((0, DIN + U), (1, 2 * U)):
        seg((l, "g12"), D, 4 * U)
        seg((l, "g0"), D, 2 * U)
        seg((l, "c12"), D, 2 * U)
        seg((l, "c0"), D, U)
    return lay, col


_BLOB_LAYOUT, BLOB_COLS = _blob_layout()

_NC_CACHE = {}


def _build_nc(t_steps: int):
    nc = bacc.Bacc("TRN2")

    xT_e = nc.declare_dram_parameter("xT", [t_steps, DIN, BN], BF16, isOutput=False)
    blob_e = nc.declare_dram_parameter("blob", [N, BLOB_COLS], BF16, isOutput=False)
    bias_e = nc.declare_dram_parameter("bias", [N, 6], F32, isOutput=False)
    h2seq_e = nc.declare_dram_parameter("h2seq", [t_steps, U, BN], BF16, isOutput=True)

    with TileContext(nc) as tc:
        with (
            tc.tile_pool(name="singles", bufs=1) as singles,
            tc.tile_pool(name="sq", bufs=2) as sq_pool,
            tc.tile_pool(name="sval", bufs=2) as sval_pool,
            tc.tile_pool(name="pq", bufs=2, space="PSUM") as pq_pool,
            tc.tile_pool(name="pqc", bufs=2, space="PSUM") as pqc_pool,
            tc.tile_pool(name="pval", bufs=2, space="PSUM") as pval_pool,
            tc.tile_pool(name="pc", bufs=2, space="PSUM") as pc_pool,
        ):
            # ---- persistent tiles ----
            blob = singles.tile([N, BLOB_COLS], BF16)
            nc.sync.dma_start(out=blob, in_=blob_e[:, :])
            bias = singles.tile([N, 6], F32)
            nc.sync.dma_start(out=bias, in_=bias_e[:, :])

            def wv(key):
                rows, c0, cols = _BLOB_LAYOUT[key]
                return blob[0:rows, c0 : c0 + cols]

            st = wv("S_T")
            s2t = wv("S2_T")
            w = {k: wv(k) for k in _BLOB_LAYOUT if isinstance(k, tuple)}

            # state tiles.  layer0 z-layout: [h1(0:U); x(U:U+DIN)].
            # layer1 z-layout: [h2(0:U); h1(U:2U)] (h2 first so its elementwise
            # ops stay at partition 0), ping-pong pair.
            zT0 = singles.tile([DIN + U, BN], BF16)
            zcT0 = singles.tile([DIN + U, BN], BF16)
            zT1_0 = singles.tile([2 * U, BN], BF16)
            zT1_1 = singles.tile([2 * U, BN], BF16)
            zT1 = (zT1_0, zT1_1)
            zcT1_0 = singles.tile([2 * U, BN], BF16)
            zcT1_1 = singles.tile([2 * U, BN], BF16)
            zcT1 = (zcT1_0, zcT1_1)
            nc.vector.memset(zT0[0:U, :], 0.0)
            nc.vector.memset(zT1_0[:, :], 0.0)
            nc.vector.memset(zT1_1[:, :], 0.0)

            def gate_project(l, zt):
                """PE wave: q12 projections + W0' term.  Returns (pqA, pqB, pval)."""
                pqA = pq_pool.tile([N, 2, 4 * U], F32, tag="pq", name="pqA")
                pqB = pq_pool.tile([N, 2, 4 * U], F32, tag="pq", name="pqB")
                for b in range(BL):
                    dst = pqA[:, b, :] if b < 2 else pqB[:, b - 2, :]
                    nc.tensor.matmul(
                        dst,
                        lhsT=zt[:, b * N : (b + 1) * N],
                        rhs=w[l, "g12"],
                        start=True,
                        stop=True,
                    )
                pval = pval_pool.tile([2 * U, BN], F32, tag="pval", name="pval")
                nc.tensor.matmul(pval, lhsT=w[l, "g0"], rhs=zt, start=True, stop=False)
                return pqA, pqB, pval

            def gate_evac(pqA, pqB):
                q12 = sq_pool.tile([N, BL, 4 * U], BF16, tag="q12", name="q12")
                nc.vector.tensor_copy(q12[:, 0:2, :], pqA)
                nc.scalar.copy(q12[:, 2:4, :], pqB)
                return q12

            def gate_diffuse(l, q12, pval):
                for b in range(BL):
                    blk = pval[:, b * N : (b + 1) * N]
                    nc.tensor.matmul(
                        blk, lhsT=q12[:, b, 0 : 2 * U], rhs=st,
                        start=False, stop=False, skip_group_check=True,
                    )
                    nc.tensor.matmul(
                        blk, lhsT=q12[:, b, 2 * U : 4 * U], rhs=s2t,
                        start=False, stop=(b == BL - 1), skip_group_check=True,
                    )

            def gate_act(l, pval):
                val = sval_pool.tile([2 * U, BN], BF16, tag="val%d" % l, name="val")
                nc.scalar.activation(
                    val, pval, mybir.ActivationFunctionType.Sigmoid,
                    bias=bias[0 : 2 * U, 3 * l : 3 * l + 1],
                )
                # um = 1-u computed directly from psum (sigmoid(-x-b))
                um = sval_pool.tile([U, BN], BF16, tag="um%d" % l, name="um")
                nc.scalar.activation(
                    um, pval[U : 2 * U, :], mybir.ActivationFunctionType.Sigmoid,
                    bias=bias[0:U, 3 * l + 2 : 3 * l + 3], scale=-1.0,
                )
                # partition-0-aligned copy of u via DMA (TensorTensor operands
                # must share a start partition); off the critical path.
                u0 = sval_pool.tile([U, BN], BF16, tag="u0%d" % l, name="u0")
                nc.sync.dma_start(out=u0, in_=val[U : 2 * U, :])
                return val, u0, um

            def cand_project(l, zct):
                pqc = pqc_pool.tile([N, BL, 2 * U], F32, tag="pqc", name="pqc")
                for b in range(BL):
                    nc.tensor.matmul(
                        pqc[:, b, :],
                        lhsT=zct[:, b * N : (b + 1) * N],
                        rhs=w[l, "c12"],
                        start=True,
                        stop=True,
                    )
                return pqc

            def cand_finish(l, zct, pqc):
                qc = sq_pool.tile([N, BL, 2 * U], BF16, tag="qc", name="qc")
                nc.vector.tensor_copy(qc, pqc)
                pc = pc_pool.tile([U, BN], F32, tag="pc", name="pc")
                nc.tensor.matmul(pc, lhsT=w[l, "c0"], rhs=zct, start=True, stop=False)
                for b in range(BL):
                    blk = pc[:, b * N : (b + 1) * N]
                    nc.tensor.matmul(
                        blk, lhsT=qc[:, b, 0:U], rhs=st,
                        start=False, stop=False, skip_group_check=True,
                    )
                    nc.tensor.matmul(
                        blk, lhsT=qc[:, b, U : 2 * U], rhs=s2t,
                        start=False, stop=(b == BL - 1), skip_group_check=True,
                    )
                c = sval_pool.tile([U, BN], BF16, tag="c", name="c")
                nc.scalar.activation(
                    c, pc, mybir.ActivationFunctionType.Tanh,
                    bias=bias[0:U, 3 * l + 1 : 3 * l + 2],
                )
                return c

            def update_pre(l, u0, h_prev):
                """e = u*h_prev; off the critical path (gpsimd), emitted early."""
                e = sval_pool.tile([U, BN], BF16, tag="e%d" % l, name="e")
                nc.gpsimd.tensor_mul(e, u0, h_prev)
                return e

            def update_post(um, c, e, h_out):
                """h_out = e + (1-u)*c  (the on-spine half of the update)."""
                m = sval_pool.tile([U, BN], BF16, tag="m", name="m")
                nc.vector.tensor_mul(m, um, c)
                nc.vector.tensor_add(h_out, e, m)

            # Software pipeline: block t runs L1's candidate for step t-1 (A),
            # then L0 gate/cand for step t (B/C), then L1's gate for step t (D).
            # L1 lags L0 by one step so both layers' dataflows are independent
            # within a block and every engine queue always has ready work.
            # Software pipeline, L1 lagging L0 by one step, with L1's gate
            # SPLIT across the block boundary: block t runs
            #   B:     L0 gate for step t        (projections ready at start)
            #   Dtail: L1 gate diffusion+act for step t-1 (projections done
            #          at the end of block t-1, so the PE never waits)
            #   A:     L1 candidate+update for step t-1
            #   C:     L0 candidate+update for step t
            #   Dproj: L1 gate projections for step t
            pend = None  # (t-1, pval1, q12_1) awaiting diffusion + act
            for blk in range(t_steps + 1):
                t = blk
                if t < t_steps:
                    nc.sync.dma_start(out=zT0[U : U + DIN, :], in_=xT_e[t])
                    nc.sync.dma_start(out=zcT0[U : U + DIN, :], in_=xT_e[t])

                    # --- B: L0 gate projections + evac ---
                    pqA0, pqB0, pval0 = gate_project(0, zT0)
                    q12_0 = gate_evac(pqA0, pqB0)

                # --- Dtail: L1 gate diffusion + activation for step t-1 ---
                if pend is not None:
                    tp, pval1, q12_1 = pend
                    gate_diffuse(1, q12_1, pval1)
                    val1, u0_1, um1 = gate_act(1, pval1)

                    # --- A: L1 candidate for step t-1 ---
                    zc1 = zcT1[tp % 2]
                    h2_prev = zT1[tp % 2][0:U, :]
                    e1 = update_pre(1, u0_1, h2_prev)
                    nc.gpsimd.tensor_mul(zc1[0:U, :], val1[0:U, :], h2_prev)

                if t < t_steps:
                    gate_diffuse(0, q12_0, pval0)

                if pend is not None:
                    pqc1 = cand_project(1, zc1)

                if t < t_steps:
                    val0, u0_0, um0 = gate_act(0, pval0)

                if pend is not None:
                    c1 = cand_finish(1, zc1, pqc1)
                    update_post(um1, c1, e1, zT1[(tp + 1) % 2][0:U, :])
                    nc.sync.dma_start(out=h2seq_e[tp], in_=zT1[(tp + 1) % 2][0:U, :])

                if t < t_steps:
                    # --- C: L0 candidate + update for step t ---
                    e0 = update_pre(0, u0_0, zT0[0:U, :])
                    nc.vector.tensor_mul(zcT0[0:U, :], val0[0:U, :], zT0[0:U, :])
                    pqc0 = cand_project(0, zcT0)
                    c0 = cand_finish(0, zcT0, pqc0)
                    update_post(um0, c0, e0, zT0[0:U, :])

                    # h1 fanout (h1 lives at partitions U:2U of layer-1 tiles);
                    # the zT1 copy gates Dproj -> DVE (low latency); the zcT1
                    # copy is only read next block -> DMA.
                    nc.vector.tensor_copy(zT1[t % 2][U : 2 * U, :], zT0[0:U, :])
                    nc.sync.dma_start(out=zcT1[t % 2][U : 2 * U, :], in_=zT0[0:U, :])

                    # --- Dproj: L1 gate projections + evac for step t ---
                    pqA1, pqB1, pval1n = gate_project(1, zT1[t % 2])
                    q12_1n = gate_evac(pqA1, pqB1)
                    pend = (t, pval1n, q12_1n)
                else:
                    pend = None

    nc.compile()
    return nc


def _prep_shared(support, W0_gate, W0_cand, W1_gate, W1_cand,
                 b0_gate, b0_cand, b1_gate, b1_cand):
    f = np.float32
    S = np.asarray(support, f)
    seg = {
        "S_T": np.ascontiguousarray(S.T),
        "S2_T": np.ascontiguousarray((2.0 * (S @ S)).T),
    }
    bias = np.zeros((N, 6), f)
    for l, (Wg, Wc, bg, bc) in enumerate(
        ((W0_gate, W0_cand, b0_gate, b0_cand), (W1_gate, W1_cand, b1_gate, b1_cand))
    ):
        Wg = np.asarray(Wg, f)
        Wc = np.asarray(Wc, f)
        g = [Wg[m::3] for m in range(3)]
        c = [Wc[m::3] for m in range(3)]
        if l == 0:
            # device z-layout for layer0 is [h(64); x(16)]
            perm = np.concatenate([np.arange(DIN, DIN + U), np.arange(DIN)])
        else:
            # device z-layout for layer1 is [h2(64); h1(64)]
            perm = np.concatenate([np.arange(U, 2 * U), np.arange(U)])
        g = [gm[perm] for gm in g]
        c = [cm[perm] for cm in c]
        seg[(l, "g12")] = np.concatenate([g[1], g[2]], axis=1)
        seg[(l, "g0")] = g[0] - g[2]
        seg[(l, "c12")] = np.concatenate([c[1], c[2]], axis=1)
        seg[(l, "c0")] = c[0] - c[2]
        bg = np.asarray(bg, f).reshape(-1)
        bias[0 : 2 * U, 3 * l] = bg
        bias[0:U, 3 * l + 1] = np.asarray(bc, f).reshape(-1)
        bias[0:U, 3 * l + 2] = -bg[U:]
    blob = np.zeros((N, BLOB_COLS), BF16_NP)
    for key, (rows, c0, cols) in _BLOB_LAYOUT.items():
        a = seg[key]
        assert a.shape == (rows, cols), (key, a.shape, rows, cols)
        blob[:rows, c0 : c0 + cols] = a.astype(BF16_NP)
    return {"blob": blob, "bias": bias}


def run_cores(inputs, t_steps=T, trace=False):
    """Build in_maps, run the SPMD kernel, return per-core h2 sequences."""
    input_seq = np.asarray(inputs["input_seq"], np.float32)
    shared = _prep_shared(
        inputs["support"], inputs["W0_gate"], inputs["W0_cand"],
        inputs["W1_gate"], inputs["W1_cand"],
        inputs["b0_gate"], inputs["b0_cand"], inputs["b1_gate"], inputs["b1_cand"],
    )
    in_maps = []
    for k in range(NCORES):
        xs = input_seq[k * BL : (k + 1) * BL, :t_steps]  # (BL, t, N, DIN)
        xT = np.ascontiguousarray(
            np.transpose(xs, (1, 3, 0, 2)).reshape(t_steps, DIN, BN)
        ).astype(BF16_NP)
        in_maps.append(dict(shared, xT=xT))
    if t_steps not in _NC_CACHE:
        _NC_CACHE[t_steps] = _build_nc(t_steps)
    nc = _NC_CACHE[t_steps]
    res = run_bass_kernel_spmd(nc, in_maps, list(range(NCORES)), trace=trace)
    return res


def finish_host(results, inputs):
    """Host tail: pick h2 at t=seq_len-1, then relu -> fc -> node max-pool."""
    W_fc = np.asarray(inputs["W_fc"], np.float32)
    b_fc = np.asarray(inputs["b_fc"], np.float32)
    seq = np.asarray(inputs["seq_lengths"]).astype(np.int64)
    out = np.empty((B, C), np.float32)
    for k in range(NCORES):
        h2seq = results[k]["h2seq"]  # (t_steps, U, BN) bf16
        t_steps = h2seq.shape[0]
        for b in range(BL):
            tb = int(min(seq[k * BL + b] - 1, t_steps - 1))
            blk = np.asarray(h2seq[tb, :, b * N : (b + 1) * N], np.float32).T  # (N, U)
            logits = np.maximum(blk, 0.0) @ W_fc + b_fc  # (N, C)
            out[k * BL + b] = logits.max(axis=0)
    return out


def kernel(**inputs):
    seq = np.asarray(inputs["seq_lengths"]).astype(np.int64)
    t_steps = int(min(T, max(1, int(seq.max()))))
    res = run_cores(inputs, t_steps=t_steps)
    return finish_host(res.results, inputs)
